# revision 1
# baseline (speedup 1.0000x reference)
"""Trainium2 Bass kernel for moe_routing (nn_CITADEL_15118284882566).

Math: the reference collapses (for qw >= 0, and max rows always containing
zeros from non-matches) to, per pair b:

    out[b] = sum_q qw[b,q] * relu( max_{l,kd} sims[b,q,l] * dw[b,l,kd]
                                   * [d_id[b,l,kd] == q_id[b,q]] )
             + dot(q_cls[b], d_cls[b])

Device strategy (data-parallel over B across 8 cores, 64 pairs/core,
processed in 16 groups of 4 pairs; partitions = 4 pairs x 32 queries):

1. DIFF2 = (d_id - q_id) + 2^-12 * dw via ONE K-stacked fp16 matmul per
   512-chunk (ids split hi/lo so all operands are fp16-exact; the dw rows
   accumulate last, so for matches the integer part cancels exactly and
   DIFF2 recovers fp16(dw) * 2^-12 EXACTLY; non-matches have |DIFF2| >= 0.999).
2. sims via 4 column-tiled matmuls (contraction over D=128 on partitions).
3. ACT copies DIFF2 * 2^20 -> fp16: matches become dw*2^8, non-matches
   saturate to +-inf.
4. DVE: (x*0)+x maps +-inf -> NaN; NaN-ignoring max-tree over kd; multiply
   by sims; NaN-ignoring reduce_max over l; relu * qw (NaN rows -> 0).
5. Final: tok sums via one-hot matmul (scaled 2^-8), cls dots via
   elementwise mult + ones matmul; host adds the two tiny outputs.
"""
import sys

sys.path.insert(0, "/opt/trn_rl_repo")

import numpy as np

B, LQ, LD, KQ, KD, D = 512, 32, 512, 1, 5, 128
NCORES = 8
BPC = B // NCORES          # 64 pairs per core
NB = 4                     # pairs per group
G = BPC // NB              # 16 groups
P = 128
JD = KD * LD               # 2560
KSTACK = 14
EPS = 2.0 ** -12
SCALE = 2.0 ** 20

_CACHED = {}


def _build_module():
    import concourse.bacc as bacc
    import concourse.mybir as mybir
    from concourse import tile

    f16 = mybir.dt.float16
    f32 = mybir.dt.float32
    Alu = mybir.AluOpType
    Act = mybir.ActivationFunctionType

    nc = bacc.Bacc("TRN2", target_bir_lowering=False, debug=False)

    dT_d = nc.dram_tensor("dT", [BPC, D, LD], f16, kind="ExternalInput")
    qT_d = nc.dram_tensor("qT", [G, D, NB * LQ], f16, kind="ExternalInput")
    lhs_d = nc.dram_tensor("lhs", [G, KSTACK, P], f16, kind="ExternalInput")
    rhs_d = nc.dram_tensor("rhs", [G, KSTACK, JD], f16, kind="ExternalInput")
    qw_d = nc.dram_tensor("qw", [G, P, 1], f32, kind="ExternalInput")
    qcT_d = nc.dram_tensor("qcT", [D, BPC], f32, kind="ExternalInput")
    dcT_d = nc.dram_tensor("dcT", [D, BPC], f32, kind="ExternalInput")
    e4s_d = nc.dram_tensor("e4s", [P, NB], f32, kind="ExternalInput")
    ones_d = nc.dram_tensor("ones", [P, 1], f32, kind="ExternalInput")

    tok_d = nc.dram_tensor("tok", [NB, G], f32, kind="ExternalOutput")
    cls_d = nc.dram_tensor("cls", [1, BPC], f32, kind="ExternalOutput")

    with tile.TileContext(nc) as tc:
        with (
            tc.tile_pool(name="sb_io", bufs=3) as sb_io,
            tc.tile_pool(name="sb_big", bufs=2) as sb_big,
            tc.tile_pool(name="sb_wk", bufs=2) as sb_wk,
            tc.tile_pool(name="sb_res", bufs=1) as sb_res,
            tc.tile_pool(name="ps_diff", bufs=1, space="PSUM") as ps_diff,
            tc.tile_pool(name="ps_s", bufs=2, space="PSUM") as ps_s,
        ):
            res = sb_res.tile([P, G], f32)

            for g in range(G):
                rhs_t = sb_io.tile([KSTACK, JD], f16, name="rhs_t")
                lhs_t = sb_io.tile([KSTACK, P], f16, name="lhs_t")
                qT_t = sb_io.tile([D, NB * LQ], f16, name="qT_t")
                dT_t = sb_io.tile([D, NB * LD], f16, name="dT_t")
                qw_t = sb_io.tile([P, 1], f32, name="qw_t")
                nc.sync.dma_start(rhs_t[:], rhs_d[g, :, :])
                nc.sync.dma_start(lhs_t[:], lhs_d[g, :, :])
                nc.sync.dma_start(qT_t[:], qT_d[g, :, :])
                for b in range(NB):
                    nc.sync.dma_start(
                        dT_t[:, b * LD:(b + 1) * LD], dT_d[g * NB + b, :, :]
                    )
                nc.sync.dma_start(qw_t[:], qw_d[g, :, :])

                diff2 = ps_diff.tile([P, JD], f32, name="diff2")
                for k in range(KD):
                    nc.tensor.matmul(
                        diff2[:, k * LD:(k + 1) * LD],
                        lhs_t[:],
                        rhs_t[:, k * LD:(k + 1) * LD],
                        start=True, stop=True,
                    )
                s_ps = ps_s.tile([P, LD], f32, name="s_ps", tag="spool")
                for b in range(NB):
                    nc.tensor.matmul(
                        s_ps[b * LQ:(b + 1) * LQ, :],
                        qT_t[:, b * LQ:(b + 1) * LQ],
                        dT_t[:, b * LD:(b + 1) * LD],
                        start=True, stop=True,
                        tile_position=(0, b * LQ),
                    )

                d2s = sb_big.tile([P, JD], f16, name="d2s")
                nc.scalar.activation(d2s[:], diff2[:], Act.Copy, bias=0.0, scale=SCALE)
                scp = sb_wk.tile([P, LD], f16, name="scp")
                nc.scalar.activation(scp[:], s_ps[:], Act.Copy, bias=0.0, scale=1.0)

                msk = sb_big.tile([P, JD], f16, name="msk")
                nc.vector.scalar_tensor_tensor(
                    msk[:], d2s[:], 0.0, d2s[:], Alu.mult, Alu.add,
                )

                t01 = sb_wk.tile([P, LD], f16, name="t01")
                t23 = sb_wk.tile([P, LD], f16, name="t23")
                nc.vector.tensor_tensor(t01[:], msk[:, 0:LD], msk[:, LD:2 * LD], Alu.max)
                nc.vector.tensor_tensor(t23[:], msk[:, 2 * LD:3 * LD], msk[:, 3 * LD:4 * LD], Alu.max)
                nc.vector.tensor_tensor(t01[:], t01[:], t23[:], Alu.max)
                dmx = sb_wk.tile([P, LD], f16, name="dmx")
                nc.vector.tensor_tensor(dmx[:], t01[:], msk[:, 4 * LD:5 * LD], Alu.max)
                prd = sb_wk.tile([P, LD], f16, name="prd")
                nc.vector.tensor_tensor(prd[:], scp[:], dmx[:], Alu.mult)
                mx = sb_wk.tile([P, 1], f32, name="mx")
                nc.vector.reduce_max(mx[:], prd[:], axis=mybir.AxisListType.X)
                # res[:, g] = max(mx, 0) * qw   (still carries the 2^8 factor)
                nc.vector.tensor_scalar(
                    res[:, g:g + 1], mx[:], 0.0, qw_t[:], Alu.max, Alu.mult,
                )

            # ---- epilogue: tok colsums + cls dots ----
            qcT_t = sb_res.tile([D, BPC], f32)
            dcT_t = sb_res.tile([D, BPC], f32)
            e4s_t = sb_res.tile([P, NB], f32)
            ones_t = sb_res.tile([P, 1], f32)
            nc.sync.dma_start(qcT_t[:], qcT_d[:])
            nc.sync.dma_start(dcT_t[:], dcT_d[:])
            nc.sync.dma_start(e4s_t[:], e4s_d[:])
            nc.sync.dma_start(ones_t[:], ones_d[:])

            cp = sb_res.tile([D, BPC], f32)
            nc.vector.tensor_tensor(cp[:], qcT_t[:], dcT_t[:], Alu.mult)

            tok_ps = ps_s.tile([NB, G], f32, name="tok_ps", tag="spool")
            # tok[r, g] = sum_q res[32r+q, g] * 2^-8   (e4s = E4 * 2^-8)
            nc.tensor.matmul(tok_ps[:], e4s_t[:], res[:], start=True, stop=True)
            cls_ps = ps_s.tile([1, BPC], f32, name="cls_ps", tag="spool")
            nc.tensor.matmul(cls_ps[:], ones_t[:], cp[:], start=True, stop=True)

            tok_sb = sb_res.tile([NB, G], f32)
            cls_sb = sb_res.tile([1, BPC], f32)
            nc.vector.tensor_copy(tok_sb[:], tok_ps[:])
            nc.vector.tensor_copy(cls_sb[:], cls_ps[:])
            nc.gpsimd.dma_start(tok_d[:], tok_sb[:])
            nc.gpsimd.dma_start(cls_d[:], cls_sb[:])

    nc.compile()
    return nc


def _prep_core_inputs(c, q_repr, q_w, q_ids, q_cls, d_repr, d_w, d_ids, d_cls):
    """Pure layout/packing for one core's 64 pairs."""
    s = slice(c * BPC, (c + 1) * BPC)
    qr = q_repr[s]          # [64, 32, 128] f32
    qw = q_w[s, :, 0]       # [64, 32]
    qi = q_ids[s, :, 0]     # [64, 32] int64
    qc = q_cls[s]           # [64, 128]
    dr = d_repr[s]          # [64, 512, 128]
    dw = d_w[s]             # [64, 512, 5]
    di = d_ids[s]           # [64, 512, 5]
    dc = d_cls[s]           # [64, 128]

    dT = np.ascontiguousarray(dr.transpose(0, 2, 1)).astype(np.float16)

    qT = np.zeros((G, D, NB * LQ), np.float16)
    lhs = np.zeros((G, KSTACK, P), np.float32)
    rhs = np.zeros((G, KSTACK, JD), np.float32)
    qww = np.zeros((G, P, 1), np.float32)

    q_hi = (qi >> 8).astype(np.float32)
    q_lo = (qi & 255).astype(np.float32)
    d_hi = (di >> 8).astype(np.float32)
    d_lo = (di & 255).astype(np.float32)
    dw16 = dw.astype(np.float16).astype(np.float32)

    E = np.zeros((NB, P), np.float32)
    for b in range(NB):
        E[b, b * LQ:(b + 1) * LQ] = 1.0

    for g in range(G):
        bb = slice(g * NB, (g + 1) * NB)
        qT[g] = np.concatenate(
            [qr[g * NB + b].T for b in range(NB)], axis=1
        ).astype(np.float16)
        lhs[g, 0:4] = 256.0 * E
        lhs[g, 4:8] = E
        lhs[g, 8] = -q_hi[bb].reshape(-1)
        lhs[g, 9] = -q_lo[bb].reshape(-1)
        lhs[g, 10:14] = EPS * E
        # kd-major flatten: [LD, KD] -> [(kd, l)]
        rhs[g, 0:4] = d_hi[bb].transpose(0, 2, 1).reshape(NB, JD)
        rhs[g, 4:8] = d_lo[bb].transpose(0, 2, 1).reshape(NB, JD)
        rhs[g, 8] = 256.0
        rhs[g, 9] = 1.0
        rhs[g, 10:14] = dw16[bb].transpose(0, 2, 1).reshape(NB, JD)
        qww[g, :, 0] = qw[bb].reshape(-1)

    e4s = np.zeros((P, NB), np.float32)
    for b in range(NB):
        e4s[b * LQ:(b + 1) * LQ, b] = 2.0 ** -8

    return {
        "dT": dT,
        "qT": qT,
        "lhs": lhs.astype(np.float16),
        "rhs": rhs.astype(np.float16),
        "qw": qww,
        "qcT": np.ascontiguousarray(qc.T).astype(np.float32),
        "dcT": np.ascontiguousarray(dc.T).astype(np.float32),
        "e4s": e4s,
        "ones": np.ones((P, 1), np.float32),
    }


def kernel(q_expert_repr, q_expert_weights, q_expert_ids, q_cls_repr,
           d_expert_repr, d_expert_weights, d_expert_ids, d_cls_repr):
    from concourse.bass_utils import run_bass_kernel_spmd

    q_repr = np.asarray(q_expert_repr, np.float32)
    q_w = np.asarray(q_expert_weights, np.float32)
    q_ids = np.asarray(q_expert_ids, np.int64)
    q_cls = np.asarray(q_cls_repr, np.float32)
    d_repr = np.asarray(d_expert_repr, np.float32)
    d_w = np.asarray(d_expert_weights, np.float32)
    d_ids = np.asarray(d_expert_ids, np.int64)
    d_cls = np.asarray(d_cls_repr, np.float32)

    if "nc" not in _CACHED:
        _CACHED["nc"] = _build_module()
    nc = _CACHED["nc"]

    in_maps = [
        _prep_core_inputs(c, q_repr, q_w, q_ids, q_cls, d_repr, d_w, d_ids, d_cls)
        for c in range(NCORES)
    ]
    rr = run_bass_kernel_spmd(nc, in_maps, core_ids=list(range(NCORES)))

    out = np.zeros((B,), np.float32)
    for c in range(NCORES):
        tok = rr.results[c]["tok"]          # [NB, G]
        cls = rr.results[c]["cls"][0]       # [BPC]
        out[c * BPC:(c + 1) * BPC] = tok.T.reshape(-1) + cls
    return out


if __name__ == "__main__":
    rng = np.random.default_rng(0)
    ins = {
        "q_expert_repr": rng.standard_normal((B, LQ, D)).astype(np.float32),
        "q_expert_weights": rng.random((B, LQ, KQ)).astype(np.float32),
        "q_expert_ids": rng.integers(0, 30522, (B, LQ, KQ)).astype(np.int64),
        "q_cls_repr": rng.standard_normal((B, D)).astype(np.float32),
        "d_expert_repr": rng.standard_normal((B, LD, D)).astype(np.float32),
        "d_expert_weights": rng.random((B, LD, KD)).astype(np.float32),
        "d_expert_ids": rng.integers(0, 30522, (B, LD, KD)).astype(np.int64),
        "d_cls_repr": rng.standard_normal((B, D)).astype(np.float32),
    }
    out = kernel(**ins)
    print("kernel out[:8]:", out[:8])
